# revision 37
# baseline (speedup 1.0000x reference)
"""Trainium2 Bass kernel for nn_ArmRGBReg (retrieval-KNN), SPMD on 8 NeuronCores.

Sharding: the 8000 lower-arm rows are x-sorted on the host and split into 8
shards of 1000 (8 blocks of up to 128 rows per core; block boundaries adapt
so every block's candidate window fits 352 slots).  Per the sharding hint,
the host gathers mesh[upper_idx]/mesh[lower_idx] (index-only work) while
sharding, so each core receives its operands pre-packed in final layout.

All per-row x-window masking is folded into the TENSOR engine: rows and
window slots are both x-sorted, so the per-row valid interval [a_i, b_i)
forms a monotone staircase in the (row, slot) matrix, and a staircase
indicator factors through a triangular matmul:
    4*[a_i <= j < b_i] = sum_k (4*tril)[k,i] * (OA - OB)[k,j]
so  key_ij = 2 l_i'.u_j' + (C-4) - |u_j'|^2 + 4*[a_i <= j < b_i]
arrives in PSUM from TWO matmuls (fp32 bilinear + fp8 staircase; the fp8
one-hots OA/OB are host-packed).  Valid keys land in [0.25, 2.75], invalid
in [-3.75, -1.25], so no vector-engine masking is needed at all.

Per block (software-pipelined, engines balanced):
  FRONT: PE: psN = ll^T@uv (fp32) += tg^T@oh (fp8 staircase).  Act copies
     psN -> SBUF (kb).
  TOPK:  DVE L1: 11 stride-interleaved groups of 32 -> top-8 via max8 (the
     x-sorted window + striding keeps per-group membership of the true
     top-50 under 8 w.h.p.); L2: 7 rounds of max8 over the 88 survivors.
     Rank pruning between rounds is offloaded: Act computes
     sgn = Sign(v8 - cur) (0 exactly at the boundary rank, verified on HW)
     and Pool multiplies cur*sgn, so extracted ranks go negative, below
     every valid key.  The NEXT block's L1 maxes are interleaved between
     rounds so the DVE never idles on the cross-engine prune chain.
  MID:  Pool: Mm = (key >= v50) 0/1 bf16 via tensor_scalar with the
     per-row rank-50 value; PE transposes Mm; Act copies to SBUF; PE
     accumulates psO = sum_dt rw^T @ Mm_T (+ -rgb_lower via identity).
  TAIL: loss = Square(psO) on Act; DMA out.
Host work is layout-only: sorting/grouping indices, gathering rows by the
given indices, packing tiles, scattering per-core outputs back to [8,8000,3].
"""

import numpy as np
import ml_dtypes

import concourse.bass as bass
import concourse.bacc as bacc
import concourse.mybir as mybir
from concourse.bass_utils import run_bass_kernel_spmd
from concourse.masks import make_identity
from concourse.tile import TileContext

V = 107778
B = 8
NU = 8000
NL = 8000
K = 50
P = 128
BC = B * 3
NBLK = 8              # row blocks per core
WIN = 3 * P           # physical window stride (384 slots)
NG = 11               # L1 stride-interleaved groups (32 slots each)
WEFF = NG * 32        # filled window slots (352); rest is pad
NS = NG * 8           # L1 survivors (88)
CC = 2.25             # negkey constant: nk = 2l'.u' + CC - |u'|^2 (valid)
F32 = mybir.dt.float32
BF16 = mybir.dt.bfloat16
FP16 = mybir.dt.float16
FP8 = mybir.dt.float8e4
Alu = mybir.AluOpType
Act = mybir.ActivationFunctionType
XMARGIN = 0.0101      # host window half-width guard


def build_graph():
    nc = bacc.Bacc()
    # fp32 params merged into one transfer; fp8 likewise (block-0 slice of
    # oh is pulled first so the first front matmul is not gated on it)
    uvll_ext = nc.declare_dram_parameter(
        "uvll", [8, 2 * NBLK * WIN + NBLK * P], FP16, isOutput=False)
    toh_ext = nc.declare_dram_parameter(
        "toh", [P, P + NBLK * WIN], FP8, isOutput=False)
    rw_ext = nc.declare_dram_parameter("rw", [P, NBLK * 3 * BC], BF16, isOutput=False)
    rl_ext = nc.declare_dram_parameter("rl", [P, NBLK * BC], FP16, isOutput=False)
    out_ext = nc.declare_dram_parameter("out", [BC, NBLK * P], F32, isOutput=True)

    with TileContext(nc) as tc:
        with (
            tc.tile_pool(name="persist", bufs=1) as pp,
            tc.tile_pool(name="work", bufs=4) as wp,
            tc.tile_pool(name="psum_n", bufs=3, space="PSUM") as pn,
            tc.tile_pool(name="psum_m", bufs=2, space="PSUM") as pm,
            tc.tile_pool(name="psum_o", bufs=2, space="PSUM") as po,
        ):
            uvll = pp.tile([8, 2 * NBLK * WIN + NBLK * P], FP16)
            nc.sync.dma_start(out=uvll[:], in_=uvll_ext[:])
            toh = pp.tile([P, P + NBLK * WIN], FP8)
            nc.gpsimd.dma_start(out=toh[:, :P + 3 * WIN],
                                in_=toh_ext[:, :P + 3 * WIN])
            nc.gpsimd.dma_start(out=toh[:, P + 3 * WIN:],
                                in_=toh_ext[:, P + 3 * WIN:])
            rw = pp.tile([P, NBLK, 3, BC], BF16)
            nc.sync.dma_start(out=rw[:], in_=rw_ext[:])
            rl = pp.tile([P, NBLK, BC], FP16)
            nc.sync.dma_start(out=rl[:], in_=rl_ext[:])
            out_sb = pp.tile([BC, NBLK * P], F32)

            ident = pp.tile([P, P], F32)
            make_identity(nc, ident[:])
            ident16 = pp.tile([P, P], BF16)
            nc.vector.tensor_copy(ident16[:], ident[:])
            identh = pp.tile([P, P], FP16)
            nc.vector.tensor_copy(identh[:], ident[:])

            # Warm-ups: trigger act-table loads (Copy/Sign/Square) and the
            # tensor-engine p-state ramp while the input DMAs land.
            awarm = pp.tile([P, 1], F32)
            nc.scalar.copy(out=awarm[:, 0:1], in_=ident[:, 0:1])
            nc.scalar.activation(out=awarm[:, 0:1], in_=ident[:, 0:1],
                                 func=Act.Sign, bias=ident[:, 1:2], scale=-1.0)
            nc.scalar.activation(out=awarm[:, 0:1], in_=ident[:, 0:1],
                                 func=Act.Square)
            warm = pm.tile([P, 3, P], BF16, tag="ptM")
            for _ in range(3):
                nc.tensor.transpose(out=warm[:, 0, :], in_=ident16[:],
                                    identity=ident16[:])

            tiles = [dict() for _ in range(NBLK)]

            def front(t):
                d = tiles[t]
                usl = slice(t * WIN, (t + 1) * WIN)
                lsl = slice(NBLK * WIN + t * P, NBLK * WIN + (t + 1) * P)
                wsl = slice(NBLK * (WIN + P) + t * WIN,
                            NBLK * (WIN + P) + (t + 1) * WIN)
                osl = slice(P + t * WIN, P + (t + 1) * WIN)
                psN = pn.tile([P, WIN], F32, tag="psN")
                # fp16 hi/lo split of the fp32 bilinear form (err ~2^-21):
                # [hi_l;lo_l].[hi_u;hi_u] + hi_l.lo_u = l.u - lo_l.lo_u
                nc.tensor.matmul(out=psN[:], lhsT=uvll[0:8, lsl],
                                 rhs=uvll[0:8, usl], start=True, stop=False)
                nc.tensor.matmul(out=psN[:], lhsT=uvll[0:4, lsl],
                                 rhs=uvll[0:4, wsl], start=False, stop=False)
                nc.tensor.matmul(out=psN[:], lhsT=toh[:, :P],
                                 rhs=toh[:, osl], start=False, stop=True)
                kb = wp.tile([P, WIN], F32, tag="kb")
                nc.scalar.copy(out=kb[:], in_=psN[:])
                d["kb"] = kb

            def l1(t):
                """Returns the block's 11 L1 max8 ops as thunks so l2() can
                issue them inside another block's cross-engine prune holes."""
                d = tiles[t]
                kbs = d["kb"][:, :WEFF].rearrange("p (w s) -> p w s", s=NG)
                lvl1 = wp.tile([P, NS], F32, tag="lvl1")
                vals = wp.tile([P, 56], F32, tag="vals")
                d["lvl1"] = lvl1
                d["vals"] = vals
                return [lambda g=g: nc.vector.max(
                    out=lvl1[:, g * 8:(g + 1) * 8], in_=kbs[:, :, g])
                    for g in range(NG)]

            OFF = ()          # rounds whose prune runs on Act+Pool

            def l2(t, filler):
                """7 extraction rounds.  Most prunes are DVE-local stt ops
                (146ns, no cross-engine latency); rounds in OFF are offloaded
                to Act (Sign(v8-cur): -1/0/+1, exact 0 at the boundary rank)
                + Pool (cur*sgn), and the DVE stall is covered by issuing the
                next block's L1 maxes (`filler`) behind the offloaded max."""
                d = tiles[t]
                vals = d["vals"]
                cur = d["lvl1"]
                nfill = len(filler)
                for r in range(7):
                    nc.vector.max(out=vals[:, r * 8:(r + 1) * 8], in_=cur[:])
                    if r < 6:
                        v8 = vals[:, r * 8 + 7:r * 8 + 8]
                        nxt = wp.tile([P, NS], F32, tag=f"cur{r % 2}")
                        if r in OFF and filler:
                            sg = wp.tile([P, NS], BF16, tag=f"sg{r % 2}")
                            nc.scalar.activation(out=sg[:], in_=cur[:],
                                                 func=Act.Sign, bias=v8,
                                                 scale=-1.0)
                            nc.gpsimd.tensor_tensor(out=nxt[:], in0=cur[:],
                                                    in1=sg[:], op=Alu.mult)
                            nf = (nfill + 1) // 2
                            for f in filler[:nf]:
                                f()
                            del filler[:nf]
                        else:
                            nc.vector.scalar_tensor_tensor(
                                out=nxt[:], in0=cur[:], scalar=v8, in1=cur[:],
                                op0=Alu.is_lt, op1=Alu.mult)
                        cur = nxt
                for f in filler:
                    f()
                del filler[:]

            def mid(t):
                d = tiles[t]
                Mm = wp.tile([P, WIN], BF16, tag="Mm")
                if t == NBLK - 1:
                    # skips the Pool hop on the exposed drain chain
                    nc.vector.tensor_scalar(out=Mm[:], in0=d["kb"][:],
                                            scalar1=d["vals"][:, 49:50],
                                            scalar2=None, op0=Alu.is_ge)
                else:
                    nc.gpsimd.tensor_scalar(out=Mm[:], in0=d["kb"][:],
                                            scalar1=d["vals"][:, 49:50],
                                            scalar2=None, op0=Alu.is_ge)
                ptM = pm.tile([P, 3, P], BF16, tag="ptM")
                for dt in range(3):
                    nc.tensor.transpose(out=ptM[:, dt, :],
                                        in_=Mm[:, dt * P:(dt + 1) * P],
                                        identity=ident16[:])
                MT = wp.tile([P, 3, P], BF16, tag="MT")
                if t >= NBLK - 2:
                    nc.vector.tensor_copy(MT[:], ptM[:])
                else:
                    nc.scalar.copy(out=MT[:], in_=ptM[:])
                psO = po.tile([BC, P], F32, tag="psO")
                # -rgb_lower first: it has no Mm dependency, so only the rw
                # matmuls sit on the drain-critical path after the mask lands
                nc.tensor.matmul(out=psO[:], lhsT=rl[:, t, :], rhs=identh[:],
                                 start=True, stop=False)
                for dt in range(3):
                    nc.tensor.matmul(out=psO[:], lhsT=rw[:, t, dt, :],
                                     rhs=MT[:, dt, :],
                                     start=False, stop=(dt == 2))
                d["psO"] = psO

            def tail(t):
                d = tiles[t]
                lsl = slice(t * P, (t + 1) * P)
                if t == NBLK - 1:
                    sqt = wp.tile([BC, P], F32, tag="sqt")
                    nc.vector.tensor_copy(sqt[:], d["psO"][:])
                    nc.vector.tensor_tensor(out=out_sb[:, lsl], in0=sqt[:],
                                            in1=sqt[:], op=Alu.mult)
                else:
                    nc.scalar.activation(out=out_sb[:, lsl], in_=d["psO"][:],
                                         func=Act.Square)
                nc.sync.dma_start(out=out_ext[:, lsl], in_=out_sb[:, lsl])

            # Software pipeline: front(t+2) | L2(t) | mid(t) | L1(t+1) |
            # tail(t-1); the DVE queue flows L1(t), L2(t), L1(t+1), ...
            front(0)
            front(1)
            for f in l1(0):
                f()
            for t in range(NBLK):
                fill = l1(t + 1) if t + 1 < NBLK else []
                l2(t, fill)
                mid(t)
                if t + 2 < NBLK:
                    front(t + 2)
                if t > 0:
                    tail(t - 1)
            tail(NBLK - 1)
    nc.compile()
    return nc


def kernel(mesh_neutral_pose, rgb, upper_idx, lower_idx, _trace=False):
    mesh = np.ascontiguousarray(np.asarray(mesh_neutral_pose, dtype=np.float32))
    rgb_np = np.asarray(rgb, dtype=np.float32)
    up = np.asarray(upper_idx).astype(np.int64)
    lo = np.asarray(lower_idx).astype(np.int64)
    lx = np.float64(mesh[lo, 0])
    ux = np.float64(mesh[up, 0])
    order = np.argsort(lx, kind="stable")
    uord = np.argsort(ux, kind="stable")
    up_s = up[uord]
    ux_s = ux[uord]
    ux_s32 = mesh[up_s, 0]          # fp32 x of sorted candidates
    thr32 = np.float32(0.01)
    # rgb in [vertex, b*3+c] layout for fast row gathers
    rgb_vc = np.ascontiguousarray(rgb_np.transpose(1, 0, 2).reshape(V, BC))

    nc = build_graph()
    in_maps = []
    slotmaps = []
    for c in range(8):
        crows = order[c * NL // 8:(c + 1) * NL // 8]
        uvll32 = np.zeros((4, NBLK * WIN + NBLK * P), np.float32)
        uv = uvll32[:, :NBLK * WIN]
        ll = uvll32[:, NBLK * WIN:]
        toh = np.zeros((P, P + NBLK * WIN), np.float32)
        toh[:, :P] = 4.0 * np.tril(np.ones((P, P), np.float32))
        ohm = toh[:, P:]
        rw = np.zeros((P, NBLK, 3, BC), ml_dtypes.bfloat16)
        rl = np.zeros((P, NBLK, BC), np.float16)
        # pad window slot: u'=0 keeps key = CC-4 < 0 (never selected)
        uv[3, :] = CC - 4.0
        smap = np.empty((NBLK, P), np.int64)
        smap.fill(-1)
        # adaptive split: up to 128 rows per block, shrinking a block when its
        # candidate window would overflow WEFF (graph pads short blocks)
        nrows = len(crows)
        bounds = []
        i = 0
        for k in range(NBLK):
            rem = NBLK - k
            j_min = max(i + 1, nrows - (rem - 1) * P)
            j = min(i + P, nrows)
            while j > j_min:
                aa, bb = lx[crows[i]], lx[crows[j - 1]]
                i0t = np.searchsorted(ux_s, aa - XMARGIN, side="left")
                i1t = np.searchsorted(ux_s, bb + XMARGIN, side="right")
                if i1t - i0t <= WEFF:
                    break
                j -= 1
            bounds.append((i, j))
            i = j
            if i >= nrows:
                bounds.extend((nrows, nrows) for _ in range(NBLK - 1 - k))
                break
        for k in range(NBLK):
            bi, bj = bounds[k]
            blk = crows[bi:bj]
            nb = len(blk)
            if nb == 0:
                continue
            smap[k, :nb] = blk
            a, b = lx[blk].min(), lx[blk].max()
            x0 = np.float32((a + b) * 0.5)
            mb = mesh[lo[blk]] - np.array([x0, 0.5, 0.5], np.float32)
            sl = slice(k * P, k * P + nb)
            ll[0:3, sl] = mb.T
            ll[3, sl] = 1.0
            i0 = np.searchsorted(ux_s, a - XMARGIN, side="left")
            i1 = np.searchsorted(ux_s, b + XMARGIN, side="right")
            if i1 - i0 > WEFF:  # last resort: trim margin candidates
                ex = i1 - i0 - WEFF
                i0 += (ex + 1) // 2
                i1 -= ex // 2
            seg = up_s[i0:i1]
            ns = len(seg)
            cu = mesh[seg] - np.array([x0, 0.5, 0.5], np.float32)
            wsl = slice(k * WIN, k * WIN + ns)
            uv[0:3, wsl] = 2.0 * cu.T
            uv[3, wsl] = (CC - 4.0) - (cu * cu).sum(1)
            # exact per-row x-interval [ai, bi) in window-local slots,
            # evaluated with the same fp32 arithmetic as the reference mask
            segx32 = ux_s32[i0:i1]
            lx32 = mesh[lo[blk], 0]
            valid = np.abs(lx32[:, None] - segx32[None, :]) < thr32
            anyv = valid.any(1)
            ai = np.where(anyv, valid.argmax(1), 0)
            bi_ = np.where(anyv, ns - valid[:, ::-1].argmax(1), 0)
            # one-hot staircase: OA[ia_j-1, j]=1 with ia_j = #{i: ai_i <= j}
            jj = np.arange(WIN)
            av = np.full(P, WIN + 1, np.int64)
            bv = np.full(P, WIN + 1, np.int64)
            av[:nb] = ai
            bv[:nb] = bi_
            ia = np.searchsorted(av, jj, side="right")
            ib = np.searchsorted(bv, jj, side="right")
            ohk = np.zeros((P, WIN), np.float32)
            mka = ia > 0
            ohk[ia[mka] - 1, jj[mka]] += 1.0
            mkb = ib > 0
            ohk[ib[mkb] - 1, jj[mkb]] -= 1.0
            ohm[:, k * WIN:(k + 1) * WIN] = ohk
            # rgb tiles: rw = window rgb/K; rl = -rgb_lower
            rwk = np.zeros((WIN, BC), np.float32)
            rwk[:ns] = rgb_vc[seg] * np.float32(1.0 / K)
            rw[:, k, :, :] = rwk.reshape(3, P, BC).transpose(1, 0, 2)
            rl[:nb, k, :] = -rgb_vc[lo[blk]]
        slotmaps.append(smap)
        # fp16 hi/lo packing: cols [uv: hi;hi | ll: hi;lo | uv-lo: lo;- ]
        hi16 = uvll32.astype(np.float16)
        lo16 = (uvll32 - hi16.astype(np.float32)).astype(np.float16)
        nuv = NBLK * WIN
        uvll16 = np.zeros((8, 2 * NBLK * WIN + NBLK * P), np.float16)
        uvll16[0:4, :nuv] = hi16[:, :nuv]
        uvll16[4:8, :nuv] = hi16[:, :nuv]
        uvll16[0:4, nuv:nuv + NBLK * P] = hi16[:, nuv:]
        uvll16[4:8, nuv:nuv + NBLK * P] = lo16[:, nuv:]
        uvll16[0:4, nuv + NBLK * P:] = lo16[:, :nuv]
        in_maps.append({
            "uvll": uvll16,
            "toh": toh.astype(ml_dtypes.float8_e4m3),
            "rw": rw.reshape(P, NBLK * 3 * BC), "rl": rl.reshape(P, NBLK * BC),
        })
    res = run_bass_kernel_spmd(nc, in_maps, core_ids=list(range(8)), trace=_trace)
    out = np.empty((B, NL, 3), np.float32)
    for c in range(8):
        o = np.asarray(res.results[c]["out"]).reshape(B, 3, NBLK, P)
        smap = slotmaps[c]
        for k in range(NBLK):
            valid = smap[k] >= 0
            rows = smap[k][valid]
            out[:, rows, :] = o[:, :, k, valid].transpose(0, 2, 1)
    if _trace:
        return out, res
    return out


# revision 38
# speedup vs baseline: 1.0491x; 1.0491x over previous
"""Trainium2 Bass kernel for nn_ArmRGBReg (retrieval-KNN), SPMD on 8 NeuronCores.

Sharding: the 8000 lower-arm rows are x-sorted on the host and split into 8
shards of 1000 (8 blocks of up to 128 rows per core; block boundaries adapt
so every block's candidate window fits 352 slots).  Per the sharding hint,
the host gathers mesh[upper_idx]/mesh[lower_idx] (index-only work) while
sharding, so each core receives its operands pre-packed in final layout.

All per-row x-window masking is folded into the TENSOR engine: rows and
window slots are both x-sorted, so the per-row valid interval [a_i, b_i)
forms a monotone staircase in the (row, slot) matrix, and a staircase
indicator factors through a triangular matmul:
    4*[a_i <= j < b_i] = sum_k (4*tril)[k,i] * (OA - OB)[k,j]
so  key_ij = 2 l_i'.u_j' + (C-4) - |u_j'|^2 + 4*[a_i <= j < b_i]
arrives in PSUM from TWO matmuls (fp32 bilinear + fp8 staircase; the fp8
one-hots OA/OB are host-packed).  Valid keys land in [0.25, 2.75], invalid
in [-3.75, -1.25], so no vector-engine masking is needed at all.

Per block (software-pipelined, engines balanced):
  FRONT: PE: psN = ll^T@uv (fp32) += tg^T@oh (fp8 staircase).  Act copies
     psN -> SBUF (kb).
  TOPK:  DVE L1: 11 stride-interleaved groups of 32 -> top-8 via max8 (the
     x-sorted window + striding keeps per-group membership of the true
     top-50 under 8 w.h.p.); L2: 7 rounds of max8 over the 88 survivors.
     Rank pruning between rounds is offloaded: Act computes
     sgn = Sign(v8 - cur) (0 exactly at the boundary rank, verified on HW)
     and Pool multiplies cur*sgn, so extracted ranks go negative, below
     every valid key.  The NEXT block's L1 maxes are interleaved between
     rounds so the DVE never idles on the cross-engine prune chain.
  MID:  Pool: Mm = (key >= v50) 0/1 bf16 via tensor_scalar with the
     per-row rank-50 value; PE transposes Mm; Act copies to SBUF; PE
     accumulates psO = sum_dt rw^T @ Mm_T (+ -rgb_lower via identity).
  TAIL: loss = Square(psO) on Act; DMA out.
Host work is layout-only: sorting/grouping indices, gathering rows by the
given indices, packing tiles, scattering per-core outputs back to [8,8000,3].
"""

import numpy as np
import ml_dtypes

import concourse.bass as bass
import concourse.bacc as bacc
import concourse.mybir as mybir
from concourse.bass_utils import run_bass_kernel_spmd
from concourse.masks import make_identity
from concourse.tile import TileContext

V = 107778
B = 8
NU = 8000
NL = 8000
K = 50
P = 128
BC = B * 3
NBLK = 8              # row blocks per core
WIN = 3 * P           # physical window stride (384 slots)
NG = 11               # L1 stride-interleaved groups (32 slots each)
WEFF = NG * 32        # filled window slots (352); rest is pad
NS = NG * 8           # L1 survivors (88)
CC = 2.25             # negkey constant: nk = 2l'.u' + CC - |u'|^2 (valid)
F32 = mybir.dt.float32
BF16 = mybir.dt.bfloat16
FP16 = mybir.dt.float16
FP8 = mybir.dt.float8e4
Alu = mybir.AluOpType
Act = mybir.ActivationFunctionType
XMARGIN = 0.0101      # host window half-width guard


def build_graph():
    nc = bacc.Bacc()
    # fp32 params merged into one transfer; fp8 likewise (block-0 slice of
    # oh is pulled first so the first front matmul is not gated on it)
    uvll_ext = nc.declare_dram_parameter(
        "uvll", [8, 2 * NBLK * WIN + NBLK * P], FP16, isOutput=False)
    toh_ext = nc.declare_dram_parameter(
        "toh", [P, P + NBLK * WIN], FP8, isOutput=False)
    rw_ext = nc.declare_dram_parameter("rw", [P, NBLK * 3 * BC], BF16, isOutput=False)
    rl_ext = nc.declare_dram_parameter("rl", [P, NBLK * BC], FP16, isOutput=False)
    out_ext = nc.declare_dram_parameter("out", [BC, NBLK * P], F32, isOutput=True)

    with TileContext(nc) as tc:
        with (
            tc.tile_pool(name="persist", bufs=1) as pp,
            tc.tile_pool(name="work", bufs=4) as wp,
            tc.tile_pool(name="psum_n", bufs=3, space="PSUM") as pn,
            tc.tile_pool(name="psum_m", bufs=2, space="PSUM") as pm,
            tc.tile_pool(name="psum_o", bufs=2, space="PSUM") as po,
        ):
            uvll = pp.tile([8, 2 * NBLK * WIN + NBLK * P], FP16)
            nc.sync.dma_start(out=uvll[:], in_=uvll_ext[:])
            toh = pp.tile([P, P + NBLK * WIN], FP8)
            nc.scalar.dma_start(out=toh[:, :P + 3 * WIN],
                                in_=toh_ext[:, :P + 3 * WIN])
            nc.scalar.dma_start(out=toh[:, P + 3 * WIN:],
                                in_=toh_ext[:, P + 3 * WIN:])
            rw = pp.tile([P, NBLK, 3, BC], BF16)
            nc.sync.dma_start(out=rw[:], in_=rw_ext[:])
            rl = pp.tile([P, NBLK, BC], FP16)
            nc.sync.dma_start(out=rl[:], in_=rl_ext[:])
            out_sb = pp.tile([BC, NBLK * P], F32)

            ident = pp.tile([P, P], F32)
            make_identity(nc, ident[:])
            ident16 = pp.tile([P, P], BF16)
            nc.vector.tensor_copy(ident16[:], ident[:])
            identh = pp.tile([P, P], FP16)
            nc.vector.tensor_copy(identh[:], ident[:])

            # Warm-ups: trigger act-table loads (Copy/Sign/Square) and the
            # tensor-engine p-state ramp while the input DMAs land.
            awarm = pp.tile([P, 1], F32)
            nc.scalar.copy(out=awarm[:, 0:1], in_=ident[:, 0:1])
            nc.scalar.activation(out=awarm[:, 0:1], in_=ident[:, 0:1],
                                 func=Act.Sign, bias=ident[:, 1:2], scale=-1.0)
            nc.scalar.activation(out=awarm[:, 0:1], in_=ident[:, 0:1],
                                 func=Act.Square)
            warm = pm.tile([P, 3, P], BF16, tag="ptM")
            for _ in range(3):
                nc.tensor.transpose(out=warm[:, 0, :], in_=ident16[:],
                                    identity=ident16[:])

            tiles = [dict() for _ in range(NBLK)]

            def front(t):
                d = tiles[t]
                usl = slice(t * WIN, (t + 1) * WIN)
                lsl = slice(NBLK * WIN + t * P, NBLK * WIN + (t + 1) * P)
                wsl = slice(NBLK * (WIN + P) + t * WIN,
                            NBLK * (WIN + P) + (t + 1) * WIN)
                osl = slice(P + t * WIN, P + (t + 1) * WIN)
                psN = pn.tile([P, WIN], F32, tag="psN")
                # fp16 hi/lo split of the fp32 bilinear form (err ~2^-21):
                # [hi_l;lo_l].[hi_u;hi_u] + hi_l.lo_u = l.u - lo_l.lo_u
                nc.tensor.matmul(out=psN[:], lhsT=uvll[0:8, lsl],
                                 rhs=uvll[0:8, usl], start=True, stop=False)
                nc.tensor.matmul(out=psN[:], lhsT=uvll[0:4, lsl],
                                 rhs=uvll[0:4, wsl], start=False, stop=False)
                nc.tensor.matmul(out=psN[:], lhsT=toh[:, :P],
                                 rhs=toh[:, osl], start=False, stop=True)
                kb = wp.tile([P, WIN], F32, tag="kb")
                nc.scalar.copy(out=kb[:], in_=psN[:])
                d["kb"] = kb

            def l1(t):
                """Returns the block's 11 L1 max8 ops as thunks so l2() can
                issue them inside another block's cross-engine prune holes."""
                d = tiles[t]
                kbs = d["kb"][:, :WEFF].rearrange("p (w s) -> p w s", s=NG)
                lvl1 = wp.tile([P, NS], F32, tag="lvl1")
                vals = wp.tile([P, 56], F32, tag="vals")
                d["lvl1"] = lvl1
                d["vals"] = vals
                return [lambda g=g: nc.vector.max(
                    out=lvl1[:, g * 8:(g + 1) * 8], in_=kbs[:, :, g])
                    for g in range(NG)]

            OFF = ()          # rounds whose prune runs on Act+Pool

            def l2(t, filler):
                """7 extraction rounds.  Most prunes are DVE-local stt ops
                (146ns, no cross-engine latency); rounds in OFF are offloaded
                to Act (Sign(v8-cur): -1/0/+1, exact 0 at the boundary rank)
                + Pool (cur*sgn), and the DVE stall is covered by issuing the
                next block's L1 maxes (`filler`) behind the offloaded max."""
                d = tiles[t]
                vals = d["vals"]
                cur = d["lvl1"]
                nfill = len(filler)
                for r in range(7):
                    nc.vector.max(out=vals[:, r * 8:(r + 1) * 8], in_=cur[:])
                    if r < 6:
                        v8 = vals[:, r * 8 + 7:r * 8 + 8]
                        nxt = wp.tile([P, NS], F32, tag=f"cur{r % 2}")
                        if r in OFF and filler:
                            sg = wp.tile([P, NS], BF16, tag=f"sg{r % 2}")
                            nc.scalar.activation(out=sg[:], in_=cur[:],
                                                 func=Act.Sign, bias=v8,
                                                 scale=-1.0)
                            nc.gpsimd.tensor_tensor(out=nxt[:], in0=cur[:],
                                                    in1=sg[:], op=Alu.mult)
                            nf = (nfill + 1) // 2
                            for f in filler[:nf]:
                                f()
                            del filler[:nf]
                        else:
                            nc.vector.scalar_tensor_tensor(
                                out=nxt[:], in0=cur[:], scalar=v8, in1=cur[:],
                                op0=Alu.is_lt, op1=Alu.mult)
                        cur = nxt
                for f in filler:
                    f()
                del filler[:]

            def mid(t):
                d = tiles[t]
                Mm = wp.tile([P, WIN], BF16, tag="Mm")
                if t == NBLK - 1:
                    # skips the Pool hop on the exposed drain chain
                    nc.vector.tensor_scalar(out=Mm[:], in0=d["kb"][:],
                                            scalar1=d["vals"][:, 49:50],
                                            scalar2=None, op0=Alu.is_ge)
                else:
                    nc.gpsimd.tensor_scalar(out=Mm[:], in0=d["kb"][:],
                                            scalar1=d["vals"][:, 49:50],
                                            scalar2=None, op0=Alu.is_ge)
                ptM = pm.tile([P, 3, P], BF16, tag="ptM")
                for dt in range(3):
                    nc.tensor.transpose(out=ptM[:, dt, :],
                                        in_=Mm[:, dt * P:(dt + 1) * P],
                                        identity=ident16[:])
                MT = wp.tile([P, 3, P], BF16, tag="MT")
                if t >= NBLK - 2:
                    nc.vector.tensor_copy(MT[:], ptM[:])
                else:
                    nc.scalar.copy(out=MT[:], in_=ptM[:])
                psO = po.tile([BC, P], F32, tag="psO")
                # -rgb_lower first: it has no Mm dependency, so only the rw
                # matmuls sit on the drain-critical path after the mask lands
                nc.tensor.matmul(out=psO[:], lhsT=rl[:, t, :], rhs=identh[:],
                                 start=True, stop=False)
                for dt in range(3):
                    nc.tensor.matmul(out=psO[:], lhsT=rw[:, t, dt, :],
                                     rhs=MT[:, dt, :],
                                     start=False, stop=(dt == 2))
                d["psO"] = psO

            def tail(t):
                d = tiles[t]
                lsl = slice(t * P, (t + 1) * P)
                if t == NBLK - 1:
                    sqt = wp.tile([BC, P], F32, tag="sqt")
                    nc.vector.tensor_copy(sqt[:], d["psO"][:])
                    nc.vector.tensor_tensor(out=out_sb[:, lsl], in0=sqt[:],
                                            in1=sqt[:], op=Alu.mult)
                else:
                    nc.scalar.activation(out=out_sb[:, lsl], in_=d["psO"][:],
                                         func=Act.Square)
                nc.sync.dma_start(out=out_ext[:, lsl], in_=out_sb[:, lsl])

            # Software pipeline: front(t+2) | L2(t) | mid(t) | L1(t+1) |
            # tail(t-1); the DVE queue flows L1(t), L2(t), L1(t+1), ...
            front(0)
            front(1)
            for f in l1(0):
                f()
            for t in range(NBLK):
                fill = l1(t + 1) if t + 1 < NBLK else []
                l2(t, fill)
                mid(t)
                if t + 2 < NBLK:
                    front(t + 2)
                if t > 0:
                    tail(t - 1)
            tail(NBLK - 1)
    nc.compile()
    return nc


def kernel(mesh_neutral_pose, rgb, upper_idx, lower_idx, _trace=False):
    mesh = np.ascontiguousarray(np.asarray(mesh_neutral_pose, dtype=np.float32))
    rgb_np = np.asarray(rgb, dtype=np.float32)
    up = np.asarray(upper_idx).astype(np.int64)
    lo = np.asarray(lower_idx).astype(np.int64)
    lx = np.float64(mesh[lo, 0])
    ux = np.float64(mesh[up, 0])
    order = np.argsort(lx, kind="stable")
    uord = np.argsort(ux, kind="stable")
    up_s = up[uord]
    ux_s = ux[uord]
    ux_s32 = mesh[up_s, 0]          # fp32 x of sorted candidates
    thr32 = np.float32(0.01)
    # rgb in [vertex, b*3+c] layout for fast row gathers
    rgb_vc = np.ascontiguousarray(rgb_np.transpose(1, 0, 2).reshape(V, BC))

    nc = build_graph()
    in_maps = []
    slotmaps = []
    for c in range(8):
        crows = order[c * NL // 8:(c + 1) * NL // 8]
        uvll32 = np.zeros((4, NBLK * WIN + NBLK * P), np.float32)
        uv = uvll32[:, :NBLK * WIN]
        ll = uvll32[:, NBLK * WIN:]
        toh = np.zeros((P, P + NBLK * WIN), np.float32)
        toh[:, :P] = 4.0 * np.tril(np.ones((P, P), np.float32))
        ohm = toh[:, P:]
        rw = np.zeros((P, NBLK, 3, BC), ml_dtypes.bfloat16)
        rl = np.zeros((P, NBLK, BC), np.float16)
        # pad window slot: u'=0 keeps key = CC-4 < 0 (never selected)
        uv[3, :] = CC - 4.0
        smap = np.empty((NBLK, P), np.int64)
        smap.fill(-1)
        # adaptive split: up to 128 rows per block, shrinking a block when its
        # candidate window would overflow WEFF (graph pads short blocks)
        nrows = len(crows)
        bounds = []
        i = 0
        for k in range(NBLK):
            rem = NBLK - k
            j_min = max(i + 1, nrows - (rem - 1) * P)
            j = min(i + P, nrows)
            while j > j_min:
                aa, bb = lx[crows[i]], lx[crows[j - 1]]
                i0t = np.searchsorted(ux_s, aa - XMARGIN, side="left")
                i1t = np.searchsorted(ux_s, bb + XMARGIN, side="right")
                if i1t - i0t <= WEFF:
                    break
                j -= 1
            bounds.append((i, j))
            i = j
            if i >= nrows:
                bounds.extend((nrows, nrows) for _ in range(NBLK - 1 - k))
                break
        for k in range(NBLK):
            bi, bj = bounds[k]
            blk = crows[bi:bj]
            nb = len(blk)
            if nb == 0:
                continue
            smap[k, :nb] = blk
            a, b = lx[blk].min(), lx[blk].max()
            x0 = np.float32((a + b) * 0.5)
            mb = mesh[lo[blk]] - np.array([x0, 0.5, 0.5], np.float32)
            sl = slice(k * P, k * P + nb)
            ll[0:3, sl] = mb.T
            ll[3, sl] = 1.0
            i0 = np.searchsorted(ux_s, a - XMARGIN, side="left")
            i1 = np.searchsorted(ux_s, b + XMARGIN, side="right")
            if i1 - i0 > WEFF:  # last resort: trim margin candidates
                ex = i1 - i0 - WEFF
                i0 += (ex + 1) // 2
                i1 -= ex // 2
            seg = up_s[i0:i1]
            ns = len(seg)
            cu = mesh[seg] - np.array([x0, 0.5, 0.5], np.float32)
            wsl = slice(k * WIN, k * WIN + ns)
            uv[0:3, wsl] = 2.0 * cu.T
            uv[3, wsl] = (CC - 4.0) - (cu * cu).sum(1)
            # exact per-row x-interval [ai, bi) in window-local slots,
            # evaluated with the same fp32 arithmetic as the reference mask
            segx32 = ux_s32[i0:i1]
            lx32 = mesh[lo[blk], 0]
            valid = np.abs(lx32[:, None] - segx32[None, :]) < thr32
            anyv = valid.any(1)
            ai = np.where(anyv, valid.argmax(1), 0)
            bi_ = np.where(anyv, ns - valid[:, ::-1].argmax(1), 0)
            # one-hot staircase: OA[ia_j-1, j]=1 with ia_j = #{i: ai_i <= j}
            jj = np.arange(WIN)
            av = np.full(P, WIN + 1, np.int64)
            bv = np.full(P, WIN + 1, np.int64)
            av[:nb] = ai
            bv[:nb] = bi_
            ia = np.searchsorted(av, jj, side="right")
            ib = np.searchsorted(bv, jj, side="right")
            ohk = np.zeros((P, WIN), np.float32)
            mka = ia > 0
            ohk[ia[mka] - 1, jj[mka]] += 1.0
            mkb = ib > 0
            ohk[ib[mkb] - 1, jj[mkb]] -= 1.0
            ohm[:, k * WIN:(k + 1) * WIN] = ohk
            # rgb tiles: rw = window rgb/K; rl = -rgb_lower
            rwk = np.zeros((WIN, BC), np.float32)
            rwk[:ns] = rgb_vc[seg] * np.float32(1.0 / K)
            rw[:, k, :, :] = rwk.reshape(3, P, BC).transpose(1, 0, 2)
            rl[:nb, k, :] = -rgb_vc[lo[blk]]
        slotmaps.append(smap)
        # fp16 hi/lo packing: cols [uv: hi;hi | ll: hi;lo | uv-lo: lo;- ]
        hi16 = uvll32.astype(np.float16)
        lo16 = (uvll32 - hi16.astype(np.float32)).astype(np.float16)
        nuv = NBLK * WIN
        uvll16 = np.zeros((8, 2 * NBLK * WIN + NBLK * P), np.float16)
        uvll16[0:4, :nuv] = hi16[:, :nuv]
        uvll16[4:8, :nuv] = hi16[:, :nuv]
        uvll16[0:4, nuv:nuv + NBLK * P] = hi16[:, nuv:]
        uvll16[4:8, nuv:nuv + NBLK * P] = lo16[:, nuv:]
        uvll16[0:4, nuv + NBLK * P:] = lo16[:, :nuv]
        in_maps.append({
            "uvll": uvll16,
            "toh": toh.astype(ml_dtypes.float8_e4m3),
            "rw": rw.reshape(P, NBLK * 3 * BC), "rl": rl.reshape(P, NBLK * BC),
        })
    res = run_bass_kernel_spmd(nc, in_maps, core_ids=list(range(8)), trace=_trace)
    out = np.empty((B, NL, 3), np.float32)
    for c in range(8):
        o = np.asarray(res.results[c]["out"]).reshape(B, 3, NBLK, P)
        smap = slotmaps[c]
        for k in range(NBLK):
            valid = smap[k] >= 0
            rows = smap[k][valid]
            out[:, rows, :] = o[:, :, k, valid].transpose(0, 2, 1)
    if _trace:
        return out, res
    return out
